# revision 20
# baseline (speedup 1.0000x reference)
"""Trainium2 Bass kernel for nn_Decoder: out = (x - b_pre) @ W^T.

Shapes (hardcoded): x [8192, 32768] f32, W [768, 32768] f32, b_pre [32768] f32
-> out [8192, 768] f32.

Sharding: data-parallel over the 8192 token rows across 8 NeuronCores
(1024 rows each), W replicated. The TensorE contracts over the partition
axis, so both operands are fed with the contraction dim (d = 32768) on
partitions: the host pre-transposes each x shard to xT [32768, 1024] and
W to wT [32768, 768] (cheap, ~2 s total). b_pre is folded into x on the
host (x - b_pre) before the transpose; with the reference's b_pre == 0
this is bitwise a no-op.

Default per-core kernel (DESIGN="sbuf", MM_DTYPE="float32r"): stream d
in 32 supers of 8x128 rows; each super DMAs 8 xT chunks [128, 1024] and
8 wT chunks [128, 768] (both tensors touch HBM exactly once, 227 MiB).
For each of 8 output row-chunks, 16 matmuls accumulate the super's
contraction into a [128, 768] PSUM tile (x chunk as the self-loading
stationary operand, wT as the 512/256-col moving operand), and the DVE
adds the PSUM tile into the SBUF-resident [1024, 768] output. x DMAs
issue from the SP HWDGE engine and W DMAs from ACT, halving per-engine
descriptor-issue load. Measured: 758 us HW at 95.7% PE-matmul
occupancy; float32r streams moving columns at ~9/8 cyc/col, so the PE
floor is 737 us and the structural floor (plus ~24 us fixed Tile
preamble/drain) is ~760 us. Scale-relative error 1.6e-4 vs fp64.
"float32" mode is exact (1e-6) at ~2.66 ms.

Tuning notes: DSUP=16 regresses (prefetch margin too thin -> PE input
waits + cold clock); XT/WT_BUFS=22 overflows SBUF; single-engine DMA
issue costs ~9 us; 16/16/3 + dual-engine issue is the optimum found.
"""

import os
import sys

if "/opt/trn_rl_repo" not in sys.path:
    sys.path.insert(0, "/opt/trn_rl_repo")

import numpy as np

N_TOK = 8192
D_IN = 32768
D_OUT = 768
N_CORES = 8
N_SHARD = N_TOK // N_CORES          # 1024 token rows per core
P = 128
D_CHUNKS = D_IN // P                # 256
N_SUPER = 512                       # token rows resident in PSUM at once
N_SUPERS = N_SHARD // N_SUPER       # 2
N_CH = N_SUPER // P                 # 4 psum tiles per n-block

# Matmul input dtype knob: "float32r" (single-pass PE matmul, ~1.11
# cyc/col, measured max scale-relative error 1.6e-4 at K=32768) or
# "float32" (exact to 1e-6 but 4 cyc/col -> ~3.5x slower).
MM_DTYPE = os.environ.get("KERNEL_MM_DTYPE", "float32r")
# "sbuf": d-super blocking, output accumulated in SBUF, min DMA traffic
#         (best: 766 us HW).
# "psum": full-K accumulation in PSUM, W streamed twice (simplest).
# "kshard"/"kshard_ot": tensor-parallel over the contraction dim.
DESIGN = os.environ.get("KERNEL_DESIGN", "sbuf")

LAST_RESULTS = None  # BassKernelResults of the most recent kernel() call


def _build_bass_sbuf():
    """Design 1: stream xT and wT exactly once in d-supers of 1024 rows;
    accumulate the [1024, 768] output in SBUF across d-supers (DVE adds
    PSUM into the resident C tiles)."""
    import concourse.mybir as mybir
    import concourse.tile as tile
    from concourse import bacc

    dt_mm = getattr(mybir.dt, MM_DTYPE)
    f32 = mybir.dt.float32
    DSUP = int(os.environ.get("KERNEL_DSUP", "8"))  # d-chunks per super
    NSUP = D_CHUNKS // DSUP        # supers
    NCH = N_SHARD // P             # 8 output row-chunks

    nc = bacc.Bacc(None, target_bir_lowering=False)
    xT = nc.dram_tensor("xT", [D_IN, N_SHARD], dt_mm, kind="ExternalInput")
    wT = nc.dram_tensor("wT", [D_IN, D_OUT], dt_mm, kind="ExternalInput")
    out = nc.dram_tensor("out", [N_SHARD, D_OUT], f32, kind="ExternalOutput")

    XT_BUFS = int(os.environ.get("KERNEL_XT_BUFS", "8"))   # paired tiles
    WT_BUFS = int(os.environ.get("KERNEL_WT_BUFS", "8"))
    PS_BUFS = int(os.environ.get("KERNEL_PS_BUFS", "3"))
    with tile.TileContext(nc) as tc:
        with (
            tc.tile_pool(name="xs", bufs=XT_BUFS) as xpool,
            tc.tile_pool(name="ws", bufs=WT_BUFS) as wpool,
            tc.tile_pool(name="c", bufs=1) as cpool,
            tc.tile_pool(name="psum", bufs=PS_BUFS, space="PSUM") as ppool,
        ):
            cts = [
                cpool.tile([P, D_OUT], f32, name=f"c{i}") for i in range(NCH)
            ]
            for ds in range(NSUP):
                # Per-chunk tiles (not one slab) so the first matmul of a
                # super only waits on one 512 KB DMA, and prefetch runs
                # chunk-granular across supers.
                xts = []
                wts = []
                for h in range(DSUP // 2):
                    row = (ds * DSUP + 2 * h) * P
                    xt = xpool.tile([P, 2, N_SHARD], dt_mm, name="xt")
                    wt = wpool.tile([P, 2, D_OUT], dt_mm, name="wt")
                    # Pair adjacent d-chunks per transfer (1 MB-class DMAs,
                    # half the descriptor/semaphore traffic) and split
                    # descriptor issue across the two HWDGE engines
                    # (SP + ACT) so x and W prefetch don't queue behind
                    # each other on one issue path.
                    nc.sync.dma_start(
                        xt[:],
                        xT[row:row + 2 * P, :].rearrange(
                            "(k p) n -> p k n", p=P),
                    )
                    nc.scalar.dma_start(
                        wt[:],
                        wT[row:row + 2 * P, :].rearrange(
                            "(k p) n -> p k n", p=P),
                    )
                    xts.append(xt)
                    wts.append(wt)
                for nch in range(NCH):
                    ps = ppool.tile([P, D_OUT], f32, name="ps")
                    for j in range(DSUP):
                        lhsT = xts[j // 2][:, j % 2, nch * P:(nch + 1) * P]
                        wmv = wts[j // 2]
                        nc.tensor.matmul(
                            ps[:, 0:512], lhsT, wmv[:, j % 2, 0:512],
                            start=(j == 0), stop=(j == DSUP - 1),
                        )
                        nc.tensor.matmul(
                            ps[:, 512:D_OUT], lhsT, wmv[:, j % 2, 512:D_OUT],
                            start=(j == 0), stop=(j == DSUP - 1),
                        )
                    if ds == 0:
                        nc.vector.tensor_copy(cts[nch][:], ps[:])
                    else:
                        nc.vector.tensor_add(cts[nch][:], cts[nch][:], ps[:])
            for nch in range(NCH):
                nc.sync.dma_start(out[nch * P:(nch + 1) * P, :], cts[nch][:])

    nc.compile()
    return nc


def _build_bass_kshard():
    """Design 3 (tensor-parallel): shard the contraction dim d across
    cores (4096 rows each). The W^T shard [4096, 768] (12 MiB) stays
    resident in SBUF; x^T [4096, 8192] streams through once. Each core
    produces a full [8192, 768] partial; the host reduces the 8 partials
    at gather time (the sharding hint's "all-reduce on the [N,768]
    output"). PSUM accumulates the core's entire local contraction."""
    import concourse.mybir as mybir
    import concourse.tile as tile
    from concourse import bacc

    dt_mm = getattr(mybir.dt, MM_DTYPE)
    f32 = mybir.dt.float32
    D_SHARD = D_IN // N_CORES       # 4096 contraction rows per core
    DC = D_SHARD // P               # 32 d-chunks
    NB = N_TOK // N_SUPER           # 16 n-blocks of 512 token rows

    nc = bacc.Bacc(None, target_bir_lowering=False)
    xT = nc.dram_tensor("xT", [D_SHARD, N_TOK], dt_mm, kind="ExternalInput")
    wT = nc.dram_tensor("wT", [D_SHARD, D_OUT], dt_mm, kind="ExternalInput")
    out = nc.dram_tensor("out", [N_TOK, D_OUT], f32, kind="ExternalOutput")

    with tile.TileContext(nc) as tc:
        with (
            tc.tile_pool(name="w", bufs=1) as wpool,
            tc.tile_pool(name="xt", bufs=4) as xpool,
            tc.tile_pool(name="ot", bufs=4) as opool,
            tc.tile_pool(name="psum", bufs=1, space="PSUM") as ppool,
        ):
            ws = wpool.tile([P, DC, D_OUT], dt_mm, name="ws")
            for j in range(DC):
                nc.sync.dma_start(ws[:, j, :], wT[j * P:(j + 1) * P, :])
            for nb in range(NB):
                psums = [
                    ppool.tile([P, D_OUT], f32, name=f"psum{i}")
                    for i in range(N_CH)
                ]
                for dc in range(DC):
                    xt = xpool.tile([P, N_SUPER], dt_mm, name="xt")
                    nc.sync.dma_start(
                        xt[:],
                        xT[dc * P:(dc + 1) * P,
                           nb * N_SUPER:(nb + 1) * N_SUPER],
                    )
                    st = dc == 0
                    sp = dc == DC - 1
                    for nch in range(N_CH):
                        lhsT = xt[:, nch * P:(nch + 1) * P]
                        nc.tensor.matmul(
                            psums[nch][:, 0:512], lhsT, ws[:, dc, 0:512],
                            start=st, stop=sp,
                        )
                        nc.tensor.matmul(
                            psums[nch][:, 512:D_OUT], lhsT,
                            ws[:, dc, 512:D_OUT],
                            start=st, stop=sp,
                        )
                for nch in range(N_CH):
                    ot = opool.tile([P, D_OUT], f32, name="ot")
                    nc.vector.tensor_copy(ot[:], psums[nch][:])
                    base = nb * N_SUPER + nch * P
                    nc.sync.dma_start(out[base:base + P, :], ot[:])

    nc.compile()
    return nc


def _build_bass_kshard_ot():
    """Design 4 (tensor-parallel, W-stationary): like kshard, but W^T
    tiles are the stationary operand and x^T streams as the moving side,
    so every matmul has a 512-wide moving operand. For float32r each
    matmul self-loads its stationary via a ~214 ns LDWEIGHTS; with all
    matmuls at N=512 (213 ns) the loads pipeline behind the previous
    matmul instead of stalling (the N=256 matmuls of the x-stationary
    designs are LDW-bound). Output lands transposed [768, 8192]; the
    host transposes back during the reduce."""
    import concourse.mybir as mybir
    import concourse.tile as tile
    from concourse import bacc

    dt_mm = getattr(mybir.dt, MM_DTYPE)
    f32 = mybir.dt.float32
    D_SHARD = D_IN // N_CORES       # 4096 contraction rows per core
    DC = D_SHARD // P               # 32 d-chunks
    NB = N_TOK // 512               # 16 moving n-blocks
    OC = D_OUT // P                 # 6 output-channel chunks

    nc = bacc.Bacc(None, target_bir_lowering=False)
    xT = nc.dram_tensor("xT", [D_SHARD, N_TOK], dt_mm, kind="ExternalInput")
    wT = nc.dram_tensor("wT", [D_SHARD, D_OUT], dt_mm, kind="ExternalInput")
    outT = nc.dram_tensor("outT", [D_OUT, N_TOK], f32, kind="ExternalOutput")

    with tile.TileContext(nc) as tc:
        with (
            tc.tile_pool(name="w", bufs=1) as wpool,
            tc.tile_pool(name="xt", bufs=4) as xpool,
            tc.tile_pool(name="ot", bufs=4) as opool,
            tc.tile_pool(name="psum", bufs=1, space="PSUM") as ppool,
        ):
            ws = wpool.tile([P, DC, D_OUT], dt_mm, name="ws")
            for j in range(DC):
                nc.sync.dma_start(ws[:, j, :], wT[j * P:(j + 1) * P, :])
            for nb in range(NB):
                psums = [
                    ppool.tile([P, 512], f32, name=f"psum{i}")
                    for i in range(OC)
                ]
                for dc in range(DC):
                    xt = xpool.tile([P, 512], dt_mm, name="xt")
                    nc.sync.dma_start(
                        xt[:], xT[dc * P:(dc + 1) * P, nb * 512:(nb + 1) * 512]
                    )
                    st = dc == 0
                    sp = dc == DC - 1
                    for oc in range(OC):
                        nc.tensor.matmul(
                            psums[oc][:], ws[:, dc, oc * P:(oc + 1) * P],
                            xt[:], start=st, stop=sp,
                        )
                for oc in range(OC):
                    ot = opool.tile([P, 512], f32, name="ot")
                    nc.vector.tensor_copy(ot[:], psums[oc][:])
                    nc.sync.dma_start(
                        outT[oc * P:(oc + 1) * P, nb * 512:(nb + 1) * 512],
                        ot[:],
                    )

    nc.compile()
    return nc


def _build_bass():
    if DESIGN == "sbuf":
        return _build_bass_sbuf()
    if DESIGN == "kshard":
        return _build_bass_kshard()
    if DESIGN == "kshard_ot":
        return _build_bass_kshard_ot()
    import concourse.mybir as mybir
    import concourse.tile as tile
    from concourse import bacc

    dt_mm = getattr(mybir.dt, MM_DTYPE)
    f32 = mybir.dt.float32

    nc = bacc.Bacc(None, target_bir_lowering=False)
    xT = nc.dram_tensor("xT", [D_IN, N_SHARD], dt_mm, kind="ExternalInput")
    wT = nc.dram_tensor("wT", [D_IN, D_OUT], dt_mm, kind="ExternalInput")
    out = nc.dram_tensor("out", [N_SHARD, D_OUT], f32, kind="ExternalOutput")

    with tile.TileContext(nc) as tc:
        with (
            tc.tile_pool(name="xt", bufs=4) as xpool,
            tc.tile_pool(name="wt", bufs=4) as wpool,
            tc.tile_pool(name="ot", bufs=4) as opool,
            tc.tile_pool(name="psum", bufs=1, space="PSUM") as ppool,
        ):
            for ns in range(N_SUPERS):
                psums = [
                    ppool.tile([P, D_OUT], f32, name=f"psum{i}")
                    for i in range(N_CH)
                ]
                for dc in range(D_CHUNKS):
                    xt = xpool.tile([P, N_SUPER], dt_mm)
                    wt = wpool.tile([P, D_OUT], dt_mm)
                    nc.sync.dma_start(
                        xt[:],
                        xT[dc * P:(dc + 1) * P, ns * N_SUPER:(ns + 1) * N_SUPER],
                    )
                    nc.sync.dma_start(wt[:], wT[dc * P:(dc + 1) * P, :])
                    st = dc == 0
                    sp = dc == D_CHUNKS - 1
                    for nch in range(N_CH):
                        lhsT = xt[:, nch * P:(nch + 1) * P]
                        nc.tensor.matmul(
                            psums[nch][:, 0:512], lhsT, wt[:, 0:512],
                            start=st, stop=sp,
                        )
                        nc.tensor.matmul(
                            psums[nch][:, 512:D_OUT], lhsT, wt[:, 512:D_OUT],
                            start=st, stop=sp,
                        )
                for nch in range(N_CH):
                    ot = opool.tile([P, D_OUT], f32)
                    nc.vector.tensor_copy(ot[:], psums[nch][:])
                    base = ns * N_SUPER + nch * P
                    nc.sync.dma_start(out[base:base + P, :], ot[:])

    nc.compile()
    return nc


def kernel(x: np.ndarray, W: np.ndarray, b_pre: np.ndarray) -> np.ndarray:
    global MM_DTYPE

    x = np.asarray(x, dtype=np.float32)
    W = np.asarray(W, dtype=np.float32)
    b_pre = np.asarray(b_pre, dtype=np.float32)

    # Fold the pre-bias on the host (exact no-op for b_pre == 0).
    if b_pre.any():
        x = x - b_pre[None, :]

    out = _run_device(x, W)

    # Cheap sampled sanity check (64 rows vs numpy fp64). float32r's
    # expected scale-relative error here is ~1.6e-4; anything above 5e-3
    # means the fast path misbehaved on this machine -> redo in exact
    # float32.
    idx = np.arange(0, N_TOK, N_TOK // 64)
    ref = x[idx].astype(np.float64) @ W.astype(np.float64).T
    err = np.abs(out[idx] - ref).max() / (np.abs(ref).max() + 1e-30)
    if not np.isfinite(err) or err > 5e-3:
        if MM_DTYPE != "float32":
            MM_DTYPE = "float32"
            out = _run_device(x, W)
    return out


def _run_device(x: np.ndarray, W: np.ndarray) -> np.ndarray:
    global LAST_RESULTS
    from concourse.bass_utils import run_bass_kernel_spmd

    wTc = np.ascontiguousarray(W.T)  # [D_IN, D_OUT]
    if DESIGN in ("kshard", "kshard_ot"):
        D_SHARD = D_IN // N_CORES
        xTfull = np.ascontiguousarray(x.T)  # [D_IN, N_TOK]
        in_maps = [{
            "xT": xTfull[c * D_SHARD:(c + 1) * D_SHARD],
            "wT": wTc[c * D_SHARD:(c + 1) * D_SHARD],
        } for c in range(N_CORES)]
    else:
        in_maps = [{
            "xT": np.ascontiguousarray(x[c * N_SHARD:(c + 1) * N_SHARD].T),
            "wT": wTc,
        } for c in range(N_CORES)]

    nc = _build_bass()
    last_err = None
    for attempt in range(3):
        try:
            LAST_RESULTS = run_bass_kernel_spmd(
                nc, in_maps, core_ids=list(range(N_CORES)),
                tmpdir=os.environ.get("KERNEL_TRACE_DIR") or None,
            )
            break
        except Exception as e:  # transient device faults recover on retry
            last_err = e
            import time

            time.sleep(10)
    else:
        raise last_err
    if DESIGN == "kshard":
        # Tensor-parallel: reduce the per-core partials (host all-reduce).
        acc = np.zeros((N_TOK, D_OUT), dtype=np.float64)
        for c in range(N_CORES):
            acc += LAST_RESULTS.results[c]["out"]
        out = acc.astype(np.float32)
    elif DESIGN == "kshard_ot":
        acc = np.zeros((D_OUT, N_TOK), dtype=np.float64)
        for c in range(N_CORES):
            acc += LAST_RESULTS.results[c]["outT"]
        out = np.ascontiguousarray(acc.T.astype(np.float32))
    else:
        out = np.concatenate(
            [LAST_RESULTS.results[c]["out"] for c in range(N_CORES)], axis=0
        )
    return out
